# revision 51
# baseline (speedup 1.0000x reference)
"""Trainium2 Bass kernel for nn_AttentionBlock (B=8, S=2048, D=512), v2.

Sharding: data-parallel over batch B across the 8 NeuronCores (attention is
per-sequence, weights replicated). Each core runs the full block on its own
[S, D] slice; no collectives.

Design (measured on HW at ~217us/core vs 292us for the v1 baseline):
  - host ships xT8 = (8*x)^T as fp8 e4m3 [D, S] plus x as bf16 [S, D]:
    no on-device x transposes and no requantize pass. q/k/v project
    straight out of xT8 with fp8 DoubleRow matmuls.
  - qT/kT live in one merged tile qk[P, KT, 2, S] at 16x scale; q+k psums
    share one [P, 2, SCW] psum pair-tile (2 banks) and drain in a single
    [P, 1024] instruction (ACT and DVE alternating by dt).
  - scores for a tt-pair accumulate into a [P, 2, SCW] psum pair; ONE
    [P, 1024] Exp drains both into the fp8 eT tile (32 exps total).
    All 4 chunks' scores+exps stream back-to-back (eT tiles stay live),
    then the per-chunk tails (attU fp8-DR + DoubleRow ones-matmul row-sum
    + residual + LN0) run, overlapping the FFN start.
  - LN0: DVE reciprocal + scalar_tensor_tensor residual (bf16 out),
    bn_stats/bn_aggr, quake rsqrt batched [P, 4], then one DVE
    tensor_scalar (x-m)*rstd per block writing bf16 onxb.
    (GPSIMD is avoided everywhere: ~4us/instr on real HW.)
  - FFN transposes ride the otherwise-idle DMA engines via
    dma_start_transpose, one [128, 2048] -> [128, 16, 128] XBAR transpose
    per 4-block group (12 total); no psum->sbuf transpose drains at all.
  - LN1/LN2 fold: W2/W3 pre-scaled by gamma on host; the -mean*colsum(W)
    rank-1 correction runs on the PE: mean-rows for two sites come from
    (-1/512 ones-col) @ t matmuls into one small psum, and a K=1 rank-1
    matmul with the host-shipped colsum row closes each site's psum
    accumulation. Two FFN sites share one [P, 2, D] psum pair-tile.
  - FFN pipeline: slot-groups of 4 blocks (s1 g | s2 g-1 | s3 g-2) with
    per-group batched bn stats and rsqrt; fp32 group staging tile and one
    output DMA per group.
  - tensor_tensor_reduce is NOT used (crashes TRN2 HW);
    activation accum_out works but bn_stats is used instead.
"""

import numpy as np
from contextlib import ExitStack

import concourse.bass as bass
import concourse.tile as tile
from concourse import bacc, mybir
from concourse.bass_utils import run_bass_kernel_spmd

# Problem constants (hardcoded per harness contract).
B, S, D = 8, 2048, 512
P = 128
NB = S // P            # 16 row blocks
KT = D // P            # 4 contraction tiles
SCW = 512              # attention s-chunk width
NSC = S // SCW         # 4 chunks
JB = SCW // P          # 4 s-blocks per chunk
GRP = 4                # FFN group size (blocks per pipeline slot-group)
NG = NB // GRP         # 4 groups per stage
EPS = 1e-5
SMSCALE = 1.0 / float(np.sqrt(D))   # BETA=1.0

F32 = mybir.dt.float32
BF16 = mybir.dt.bfloat16
F8 = mybir.dt.float8e4
I32 = mybir.dt.int32
AF = mybir.ActivationFunctionType
ALU = mybir.AluOpType
DR = mybir.MatmulPerfMode.DoubleRow
RSQRT_MAGIC = 0x5F3759DF
# fp8 scaling: x ships as 8*x (both as xT8), W{q,k,v} as 512*W; projection
# psums carry 4096x; qT/kT/v all store 16x values (RS rescale), so softmax
# numerator and the 16x ones-denominator cancel exactly.
QS = 16.0
XS = 8.0
WS = 512.0
RS = QS / (XS * WS)
ESC = SMSCALE / (QS * QS)

WNAMES = ["Wq", "Wk", "Wv", "W1", "W2", "W3"]


def _bcast_ap(ap, parts):
    """[D] dram AP -> [parts, D] AP broadcast along partitions."""
    return bass.AP(tensor=ap.tensor, offset=ap.offset, ap=[[0, parts]] + ap.ap)


def _emit(ctx, tc, cfg, loop_n=1, phases=3):
    nc = tc.nc
    present = cfg["present"]

    # ---- DRAM I/O ----
    x = nc.dram_tensor("x", [S, D], BF16, kind="ExternalInput").ap()
    xT8 = nc.dram_tensor("xT8", [D, S], F8, kind="ExternalInput").ap()
    w_ap = {n: nc.dram_tensor(n, [D, D],
                              F8 if n in ("Wq", "Wk", "Wv") else BF16,
                              kind="ExternalInput").ap()
            for n in WNAMES}
    vec_ap = {}
    for n in ["w2s", "w3s"]:
        vec_ap[n] = nc.dram_tensor(n, [D], BF16, kind="ExternalInput").ap()
    for n in ["bq", "bk", "bv", "b1", "bb2", "bb3", "ln0_g", "ln0_b"]:
        if n in present:
            vec_ap[n] = nc.dram_tensor(n, [D], F32, kind="ExternalInput").ap()
    out = nc.dram_tensor("out", [S, D], F32, kind="ExternalOutput").ap()

    # ---- pools ----
    consts = ctx.enter_context(tc.tile_pool(name="consts", bufs=1))
    wpool = ctx.enter_context(tc.tile_pool(name="wpool", bufs=6))
    bigp = ctx.enter_context(tc.tile_pool(name="big", bufs=1))
    xep = ctx.enter_context(tc.tile_pool(name="xe", bufs=4))
    xld = ctx.enter_context(tc.tile_pool(name="xld", bufs=4))
    work = ctx.enter_context(tc.tile_pool(name="work", bufs=4))
    glp = ctx.enter_context(tc.tile_pool(name="glp", bufs=2))
    soutp = ctx.enter_context(tc.tile_pool(name="soutp", bufs=2))
    ttp = ctx.enter_context(tc.tile_pool(name="ttp", bufs=8))
    small = ctx.enter_context(tc.tile_pool(name="small", bufs=4))
    psb = ctx.enter_context(tc.tile_pool(name="psb", bufs=2, space="PSUM"))
    psa = ctx.enter_context(tc.tile_pool(name="psa", bufs=1, space="PSUM"))
    pss = ctx.enter_context(tc.tile_pool(name="pss", bufs=1, space="PSUM"))

    # psum pair-tile cycler: rotates over 3 [P,2,512] tiles (psb holds 2,
    # psa 1) so every producer->drain pipeline runs 3 deep instead of 2
    _pcycle = [0]

    def pair_alloc(nm):
        _pcycle[0] += 1
        if _pcycle[0] % 3 == 0:
            return psa.tile([P, 2, SCW], F32, tag="mm2", name=nm)
        return psb.tile([P, 2, SCW], F32, tag="mm", name=nm)

    # ---- constants ----
    ones_f = consts.tile([P, 4], F32)
    nc.vector.memset(ones_f[:], QS)
    ones_q = consts.tile([P, 2], F8)
    nc.vector.tensor_copy(ones_q[:], ones_f[:, 0:2])
    ones_q4 = consts.tile([P, 2, 2], F8)
    nc.vector.tensor_copy(ones_q4[:], ones_f[:].rearrange("p (a b) -> p a b",
                                                          a=2))
    onescol = consts.tile([P, 1], BF16)
    nc.vector.memset(onescol[:], -1.0 / D)
    w2sr = consts.tile([1, D], BF16)
    w3sr = consts.tile([1, D], BF16)
    warm = consts.tile([P, 2], F32)
    nc.scalar.activation(warm[:], ones_f[:, 0:2], AF.Exp)
    eps_ap = consts.tile([P, 1], F32)
    nc.vector.memset(eps_ap[:], EPS)

    pp_bias = {}
    for n in ["bq", "bk"]:
        if n in present:
            t = consts.tile([P, KT], F32, tag=f"pp_{n}", name=f"pp_{n}")
            pp_bias[n] = t
    bc_tile = {}
    for n in ["bv", "b1", "bb2", "bb3", "ln0_g", "ln0_b"]:
        if n in present:
            t = consts.tile([P, D], F32, tag=f"bc_{n}", name=f"bc_{n}")
            bc_tile[n] = t

    def load_const_vecs():
        nc.sync.dma_start(w2sr[:], bass.AP(tensor=vec_ap["w2s"].tensor,
                                           offset=vec_ap["w2s"].offset,
                                           ap=[[0, 1]] + vec_ap["w2s"].ap))
        nc.sync.dma_start(w3sr[:], bass.AP(tensor=vec_ap["w3s"].tensor,
                                           offset=vec_ap["w3s"].offset,
                                           ap=[[0, 1]] + vec_ap["w3s"].ap))
        for n, t in pp_bias.items():
            nc.sync.dma_start(t[:], vec_ap[n].rearrange("(kt p) -> p kt", p=P))
        for n, t in bc_tile.items():
            nc.sync.dma_start(t[:], _bcast_ap(vec_ap[n], P))

    # ---- persistent per-sequence tensors ----
    xT8sb = bigp.tile([P, KT, S], F8, tag="xT8sb")
    qk = bigp.tile([P, KT, 2, S], F8, tag="qk")
    vt = bigp.tile([P, NB, D], F8, tag="v")
    onxb_t = [bigp.tile([P, 4, D], BF16, tag=f"onxb{i}", name=f"onxb{i}")
              for i in range(NB // 4)]

    def onxb_ap(n):
        return onxb_t[n // 4][:, n % 4, :]

    def load_w(name):
        dt_ = F8 if name in ("Wq", "Wk", "Wv") else BF16
        wt = wpool.tile([P, KT, D], dt_, tag="w", name=f"w_{name}")
        nc.sync.dma_start(wt[:], w_ap[name].rearrange("(kt p) d -> p kt d", p=P))
        return wt

    def emit_rsqrt(dst, src_ap, n):
        """dst[P,n] = 1/sqrt(src + EPS), DVE-only quake + 2 Newton steps."""
        vps = small.tile([P, n], F32, tag=f"rsq_v{n}", name="rsq_v")
        nc.vector.tensor_scalar_add(vps[:], src_ap, EPS)
        nc.vector.tensor_scalar(dst.bitcast(I32), vps[:].bitcast(I32),
                                1, None, op0=ALU.arith_shift_right)
        nc.vector.tensor_scalar(dst.bitcast(I32), dst.bitcast(I32),
                                -1, RSQRT_MAGIC, op0=ALU.mult, op1=ALU.add)
        t2 = small.tile([P, n], F32, tag=f"rsq_t{n}", name="rsq_t")
        for _ in range(3):
            nc.vector.tensor_tensor(t2[:], dst, dst, op=ALU.mult)
            nc.vector.tensor_tensor(t2[:], t2[:], vps[:], op=ALU.mult)
            nc.vector.tensor_scalar(t2[:], t2[:], -0.5, 1.5,
                                    op0=ALU.mult, op1=ALU.add)
            nc.vector.tensor_tensor(dst, dst, t2[:], op=ALU.mult)

    def emit_rsqrt_act(dst, src_ap):
        """dst = 1/sqrt(src+EPS) via ACT Ln -> Exp(-0.5*). Both funcs live in
        the natural_log_exp table set together with the softmax Exp, so this
        costs no activation-table reload inside the attention region."""
        lnv = small.tile([P, dst.shape[-1]], F32, tag="lnv", name="lnv")
        nc.scalar.activation(lnv[:], src_ap, AF.Ln, bias=eps_ap[:])
        nc.scalar.activation(dst, lnv[:], AF.Exp, scale=-0.5)

    # ================= Phase 1: projections off host-shipped xT8 ======
    if loop_n > 1:
        loop_cm = tc.For_i(0, loop_n, 1)
        loop_cm.__enter__()

    xT8_src = xT8.rearrange("(kt p) s -> p kt s", p=P)
    nc.sync.dma_start(xT8sb[:, :, 0:SCW], xT8_src[:, :, 0:SCW])
    wq = load_w("Wq")
    wk = load_w("Wk")
    nc.sync.dma_start(xT8sb[:, :, SCW:S], xT8_src[:, :, SCW:S])
    wv = load_w("Wv")
    xrgs = []
    for sc in range(NSC):
        xrg = xld.tile([P, JB, D], BF16, tag="xld", name="xrg")
        n0 = sc * JB
        nc.sync.dma_start(
            xrg[:], x[n0 * P:(n0 + JB) * P, :].rearrange(
                "(a p) d -> p a d", p=P))
        xrgs.append(xrg)
    w1 = load_w("W1")   # host: W1 + I (and ln0_g fold when present)
    w2 = load_w("W2")   # host: diag(ln1_g) @ W2
    w3 = load_w("W3")   # host: diag(ln2_g) @ W3
    load_const_vecs()

    def phase1_chunk(sc):
        cs = slice(sc * SCW, (sc + 1) * SCW)
        for dt in range(KT):
            pqk = pair_alloc("pqk")
            for kt in range(0, KT, 2):
                nc.tensor.matmul(pqk[:, 0, :],
                                 wq[:, kt:kt + 2, dt * P:(dt + 1) * P],
                                 xT8sb[:, kt:kt + 2, cs], start=(kt == 0),
                                 stop=(kt == KT - 2), perf_mode=DR)
            for kt in range(0, KT, 2):
                nc.tensor.matmul(pqk[:, 1, :],
                                 wk[:, kt:kt + 2, dt * P:(dt + 1) * P],
                                 xT8sb[:, kt:kt + 2, cs], start=(kt == 0),
                                 stop=(kt == KT - 2), perf_mode=DR)
            if "bq" in pp_bias or "bk" in pp_bias:
                nc.scalar.activation(qk[:, dt, 0, cs], pqk[:, 0, :],
                                     AF.Identity, scale=RS,
                                     bias=pp_bias["bq"][:, dt:dt + 1])
                nc.vector.scalar_tensor_tensor(
                    qk[:, dt, 1, cs], pqk[:, 1, :], RS,
                    pp_bias["bk"][:, dt:dt + 1].to_broadcast([P, SCW]),
                    op0=ALU.mult, op1=ALU.add)
            elif dt % 2 == 0:
                nc.scalar.activation(qk[:, dt, :, cs], pqk[:],
                                     AF.Identity, scale=RS)
            else:
                nc.vector.tensor_scalar_mul(qk[:, dt, :, cs], pqk[:], RS)
        for j in range(0, JB, 2):
            n = sc * JB + j
            pv2 = pair_alloc("pv2")
            for h in range(2):
                for kt in range(0, KT, 2):
                    nc.tensor.matmul(pv2[:, h, :],
                                     xT8sb[:, kt:kt + 2,
                                           (n + h) * P:(n + h + 1) * P],
                                     wv[:, kt:kt + 2, :], start=(kt == 0),
                                     stop=(kt == KT - 2), perf_mode=DR)
            if "bv" in bc_tile:
                for h in range(2):
                    nc.vector.scalar_tensor_tensor(
                        vt[:, n + h, :], pv2[:, h, :], RS,
                        bc_tile["bv"][:], op0=ALU.mult, op1=ALU.add)
            elif j % 4 == 0:
                nc.scalar.activation(vt[:, n:n + 2, :], pv2[:],
                                     AF.Identity, scale=RS)
            else:
                nc.vector.tensor_scalar_mul(vt[:, n:n + 2, :], pv2[:], RS)

    # ================= FFN machinery ==================================
    t1s, t2s, t3s = {}, {}, {}
    rstd1g, rstd2g = {}, {}
    dummy = consts.tile([P, D], BF16, tag="dummy", name="dummy")

    def tpose_group(store, g, src_group):
        """One DMA-transpose for 4 blocks: src [P, 4*D] -> [P, 4*KT, P];
        block i's [P, KT, P] t-tile lives at [:, i*KT:(i+1)*KT, :]."""
        t = ttp.tile([P, GRP * KT, P], BF16, tag="tT", name="tT")
        nc.sync.dma_start_transpose(
            t[:], src_group.rearrange("p a d -> p (a d)"))
        store[g] = t

    def t_block(store, g, i):
        return store[g][:, i * KT:(i + 1) * KT, :]

    def stats_emit(gl, mvf, i):
        # per-site bn stats; variance lands in mvf[:, i, 1]
        st = small.tile([P, 6], F32, tag="fst", name="fst")
        nc.vector.bn_stats(st[:], gl)
        nc.vector.bn_aggr(mvf[:, i, :], st[:])

    def stats_batch(g, store, tagn, mvf):
        rstd = small.tile([P, GRP], F32, tag=tagn, name="rstd")
        emit_rsqrt(rstd[:], mvf[:, :, 1], GRP)
        store[g] = rstd

    def mean_rows2(store, g, i):
        # -mean rows for sites i, i+1 packed in one small psum tile
        pmt = pss.tile([1, 2, P], F32, tag="pmT", name="pmT")
        for h in range(2):
            ts = t_block(store, g, i + h)
            for kt in range(KT):
                nc.tensor.matmul(pmt[:, h, :], onescol[:], ts[:, kt, :],
                                 start=(kt == 0), stop=(kt == KT - 1))
        mT2 = small.tile([1, 2, P], BF16, tag="mT", name="mT")
        nc.scalar.copy(mT2[:], pmt[:])
        return mT2

    _pmpair = [None]

    def ffn_pm_half():
        # two FFN sites share one [P,2,D] psum pair-tile (one bank each)
        if _pmpair[0] is None:
            _pmpair[0] = pair_alloc("pmf")
            return _pmpair[0][:, 0, :]
        h = _pmpair[0][:, 1, :]
        _pmpair[0] = None
        return h

    def ffn_mm2(store, g, i, w, wsr):
        # sites i, i+1: hoisted mean-rows, then W-matmuls + rank-1 each
        mT2 = mean_rows2(store, g, i)
        pmpair = pair_alloc("pmf2")
        pms = []
        for h in range(2):
            ts = t_block(store, g, i + h)
            pm = pmpair[:, h, :]
            for kt in range(KT):
                nc.tensor.matmul(pm, ts[:, kt, :], w[:, kt, :],
                                 start=(kt == 0), stop=False)
            nc.tensor.matmul(pm, mT2[:, h, :], wsr[:], start=False, stop=True)
            pms.append(pm)
        return pms

    def ffn_mm_full(tsrc, w):
        pm = ffn_pm_half()
        for kt in range(KT):
            nc.tensor.matmul(pm, tsrc[:, kt, :], w[:, kt, :],
                             start=(kt == 0), stop=(kt == KT - 1))
        return pm

    def ffn_s1(g, i, glg, mvf):
        pm1 = ffn_mm_full(t_block(t1s, g, i), w1)
        gl = glg[:, i, :]
        if "b1" in bc_tile:
            pre = work.tile([P, D], F32, tag="work", name="pre")
            nc.vector.tensor_add(pre[:], pm1, bc_tile["b1"][:])
            nc.scalar.activation(gl, pre[:], AF.Gelu)
        else:
            nc.scalar.activation(gl, pm1, AF.Gelu)
        stats_emit(gl, mvf, i)

    def ffn_s2pair(g, i, glg, rstd1, mvf):
        pms = ffn_mm2(t2s, g, i, w2, w2sr)
        for h in range(2):
            n = g * GRP + i + h
            pre2 = work.tile([P, D], F32, tag="work", name="pre2")
            nc.vector.scalar_tensor_tensor(pre2[:], pms[h],
                                           rstd1[:, i + h:i + h + 1],
                                           onxb_ap(n), op0=ALU.mult,
                                           op1=ALU.add)
            if "bb2" in bc_tile:
                nc.vector.tensor_add(pre2[:], pre2[:], bc_tile["bb2"][:])
            gl2 = glg[:, i + h, :]
            nc.scalar.activation(gl2, pre2[:], AF.Gelu)
            stats_emit(gl2, mvf, i + h)

    def ffn_s3pair(g, i, soutg, rstd2):
        pms = ffn_mm2(t3s, g, i, w3, w3sr)
        for h in range(2):
            ot = soutg[:, i + h, :]
            if "bb3" in bc_tile:
                nc.vector.scalar_tensor_tensor(ot, pms[h],
                                               rstd2[:, i + h:i + h + 1],
                                               bc_tile["bb3"][:],
                                               op0=ALU.mult, op1=ALU.add)
            else:
                nc.scalar.mul(ot, pms[h], rstd2[:, i + h:i + h + 1])

    def ffn_group(g):
        # s1 over blocks of group g, s2 over g-1, s3 over g-2
        if g < NG:
            mvf = small.tile([P, GRP, 2], F32, tag="mvf1", name="mvf1")
            glg = glp.tile([P, GRP, D], BF16, tag="gl1", name="glg1")
            for i in range(GRP):
                ffn_s1(g, i, glg, mvf)
            tpose_group(t2s, g, glg[:])
            del t1s[g]
            stats_batch(g, rstd1g, "rstd1", mvf)
        if 0 <= g - 1 < NG:
            mvf = small.tile([P, GRP, 2], F32, tag="mvf2", name="mvf2")
            glg = glp.tile([P, GRP, D], BF16, tag="gl2", name="glg2")
            for i in range(0, GRP, 2):
                ffn_s2pair(g - 1, i, glg, rstd1g[g - 1], mvf)
            tpose_group(t3s, g - 1, glg[:])
            del t2s[g - 1]
            stats_batch(g - 1, rstd2g, "rstd2", mvf)
        if 0 <= g - 2 < NG:
            soutg = soutp.tile([P, GRP, D], F32, tag="sout", name="soutg")
            for i in range(0, GRP, 2):
                ffn_s3pair(g - 2, i, soutg, rstd2g[g - 2])
            n0 = (g - 2) * GRP
            nc.sync.dma_start(
                out[n0 * P:(n0 + GRP) * P, :].rearrange("(a p) d -> p a d",
                                                        p=P),
                soutg[:])
            del t3s[g - 2]

    # ================= Phase 2: attention + LN0 =======================
    # software-pipelined chunks: scores+exp of chunk c+1 are emitted before
    # the attU of chunk c so the PE never drains while ACT works the exps
    def emit_scores_part(eT, sc, tp0, tp1):
        cs = slice(sc * SCW, (sc + 1) * SCW)
        for tp in range(tp0, tp1):
            pm2 = pair_alloc("pms")
            for h in range(2):
                tt = 2 * tp + h
                for kt in range(0, KT, 2):
                    nc.tensor.matmul(pm2[:, h, :],
                                     qk[:, kt:kt + 2, 1, tt * P:(tt + 1) * P],
                                     qk[:, kt:kt + 2, 0, cs],
                                     start=(kt == 0), stop=(kt == KT - 2),
                                     perf_mode=DR)
            nc.scalar.activation(eT[:, 2 * tp:2 * tp + 2, :], pm2[:],
                                 AF.Exp, scale=ESC)

    def emit_chunk_tail(sc, eT):
        xrs = [xrgs[sc][:, j, :] for j in range(JB)]
        onxrs = []
        papair = [None]
        for j in range(JB):
            if papair[0] is None:
                papair[0] = pair_alloc("pa")
                pa = papair[0][:, 0, :]
            else:
                pa = papair[0][:, 1, :]
                papair[0] = None
            if phases != 22:
                psm = pss.tile([P, 2], F32, tag="sm", name="psm")
            for tt in range(0, NB, 2):
                nc.tensor.matmul(pa,
                                 eT[:, tt:tt + 2, j * P:(j + 1) * P],
                                 vt[:, tt:tt + 2, :], start=(tt == 0),
                                 stop=(tt == NB - 2), perf_mode=DR)
                if phases == 22:
                    continue
                nc.tensor.matmul(psm[:], eT[:, tt:tt + 2, j * P:(j + 1) * P],
                                 ones_q4[:], start=(tt == 0),
                                 stop=(tt == NB - 2), perf_mode=DR)
            # drain promptly: frees the single psm bank and the pa bank
            rcp = small.tile([P, 1], F32, tag="rcp", name="rcp")
            if phases == 22:
                nc.vector.memset(rcp[:], 1.0)
            else:
                nc.vector.reciprocal(rcp[:], psm[:, 0:1])
            onxr = work.tile([P, D], BF16, tag="wkb", name="onxr")
            nc.vector.scalar_tensor_tensor(onxr[:], pa, rcp[:],
                                           xrs[j],
                                           op0=ALU.mult, op1=ALU.add)
            onxrs.append(onxr)
        if phases in (22, 23):
            return
        mvg = small.tile([P, JB, 2], F32, tag="mvg", name="mvg")
        for j in range(JB):
            st = small.tile([P, 6], F32, tag="bst", name="st")
            nc.vector.bn_stats(st[:], onxrs[j][:])
            nc.vector.bn_aggr(mvg[:, j, :], st[:])
        rstd0 = small.tile([P, JB], F32, tag="rstd0", name="rstd0")
        emit_rsqrt(rstd0[:], mvg[:, :, 1], JB)
        for j in range(JB):
            n = sc * JB + j
            nc.vector.tensor_scalar(onxb_ap(n), onxrs[j][:],
                                    mvg[:, j, 0:1], rstd0[:, j:j + 1],
                                    op0=ALU.subtract, op1=ALU.mult)
            if "ln0_g" in bc_tile:
                nc.vector.tensor_mul(onxb_ap(n), onxb_ap(n),
                                     bc_tile["ln0_g"][:])
            if "ln0_b" in bc_tile:
                nc.vector.tensor_add(onxb_ap(n), onxb_ap(n),
                                     bc_tile["ln0_b"][:])
        # chunk sc == onxb group sc: one group transpose for FFN s1
        tpose_group(t1s, sc, onxb_t[sc][:])

    # chunk-0 scores only need chunk-c k-blocks: interleave their emission
    # with the phase-1 chunks so the exp stream starts ~1/4 into phase 1
    att = phases >= 2 or phases in (21, 22, 23)
    eTs = {}
    if att:
        for sc in range(NSC):
            eTs[sc] = xep.tile([P, NB, SCW], F8, tag="eT", name="eT")
    for sc in range(NSC):
        phase1_chunk(sc)
        if att:
            emit_scores_part(eTs[0], 0, 2 * sc, 2 * sc + 2)
    # tails have no ACT work (exps all live in the scores region): tails
    # 0/1 slot between later chunks' scores, filling exp-bound PE/DVE gaps;
    # FFN groups then trail the remaining tails (gelus stay after all exps
    # so the kernel still needs only two activation-table loads)
    tails = phases >= 2 or phases in (22, 23)
    if att:
        emit_scores_part(eTs[1], 1, 0, NB // 2)
    if tails:
        emit_chunk_tail(0, eTs.pop(0))
    if att:
        emit_scores_part(eTs[2], 2, 0, NB // 2)
    if tails:
        emit_chunk_tail(1, eTs.pop(1))
    if att:
        emit_scores_part(eTs[3], 3, 0, NB // 2)
    if tails:
        emit_chunk_tail(2, eTs.pop(2))
        if phases == 3:
            ffn_group(0)
        emit_chunk_tail(3, eTs.pop(3))
        if phases == 3:
            ffn_group(1)

    # ================= Phase 3: FFN drain =============================
    if phases == 3:
        for g in range(2, NG + 2):
            ffn_group(g)
    if phases != 3:
        # timing-ablation builds: emit a dummy out store so the output
        # tensor exists
        zt = work.tile([P, D], F32, tag="work", name="zt")
        nc.vector.memset(zt[:], 0.0)
        for n in range(NB):
            nc.sync.dma_start(out[n * P:(n + 1) * P, :], zt[:])
    if loop_n > 1:
        loop_cm.__exit__(None, None, None)


def build_nc(cfg, loop_n=1, phases=3):
    nc = bacc.Bacc("TRN2", target_bir_lowering=False, debug=False)
    with tile.TileContext(nc) as tc:
        with ExitStack() as ctx:
            _emit(ctx, tc, cfg, loop_n=loop_n, phases=phases)
    nc.compile()
    return nc


def prepare(inputs):
    """Host-side folding; returns (cfg, common inputs w/o x, per-core extra)."""
    f32 = np.float32
    import ml_dtypes
    bf16 = ml_dtypes.bfloat16
    fp8 = ml_dtypes.float8_e4m3

    ln0_g = np.asarray(inputs["ln0_g"], f32)
    ln0_b = np.asarray(inputs["ln0_b"], f32)
    ln1_g = np.asarray(inputs["ln1_g"], f32)
    ln1_b = np.asarray(inputs["ln1_b"], f32)
    ln2_g = np.asarray(inputs["ln2_g"], f32)
    ln2_b = np.asarray(inputs["ln2_b"], f32)

    # device computes z = pure LN0; fold gamma into W1' = diag(g)(W1 + I)
    W1p = (ln0_g[:, None] * (np.asarray(inputs["W1"], f32)
                             + np.eye(D, dtype=f32))).astype(bf16)
    W2p = (ln1_g[:, None] * np.asarray(inputs["W2"], f32)).astype(bf16)
    W3p = (ln2_g[:, None] * np.asarray(inputs["W3"], f32)).astype(bf16)
    w2s = W2p.astype(np.float64).sum(0).astype(bf16)
    w3s = W3p.astype(np.float64).sum(0).astype(bf16)
    bb2 = (ln1_b.astype(np.float64) @ np.asarray(inputs["W2"], np.float64)
           + np.asarray(inputs["b2"], np.float64)).astype(f32)
    bb3 = (ln2_b.astype(np.float64) @ np.asarray(inputs["W3"], np.float64)
           + np.asarray(inputs["b3"], np.float64)).astype(f32)

    ws = np.float32(WS)
    common = {
        "Wq": np.ascontiguousarray((np.asarray(inputs["Wq"], f32) * ws).astype(fp8)),
        "Wk": np.ascontiguousarray((np.asarray(inputs["Wk"], f32) * ws).astype(fp8)),
        "Wv": np.ascontiguousarray((np.asarray(inputs["Wv"], f32) * ws).astype(fp8)),
        "W1": np.ascontiguousarray(W1p),
        "W2": np.ascontiguousarray(W2p),
        "W3": np.ascontiguousarray(W3p),
        "w2s": np.ascontiguousarray(w2s),
        "w3s": np.ascontiguousarray(w3s),
    }
    present = set()
    for name, val in [("bq", inputs["bq"]), ("bk", inputs["bk"]),
                      ("bv", inputs["bv"]), ("b1", inputs["b1"]),
                      ("bb2", bb2), ("bb3", bb3)]:
        val = np.asarray(val, f32)
        if np.any(val != 0.0):
            if name in ("bq", "bk", "bv"):
                val = val * np.float32(QS)
            if name == "b1":
                # device h1-pre comes from onxb @ W1p (gamma folded); the
                # b-fold for ln0_b rides bb-style, b1 adds directly
                pass
            common[name] = np.ascontiguousarray(val)
            present.add(name)
    # ln0_b: out_nxt = z*g + b; h1pre = out_nxt @ (I+W1) = z@W1p + b@(I+W1)
    if np.any(ln0_b != 0.0):
        b1fold = (ln0_b.astype(np.float64)
                  @ (np.eye(D) + np.asarray(inputs["W1"], np.float64))
                  ).astype(f32)
        common["b1"] = np.ascontiguousarray(
            common.get("b1", np.zeros(D, f32)) + b1fold)
        present.add("b1")
        # the s2 residual uses onxb (= z); the true residual is z*g + b
        common["ln0_g"] = np.ascontiguousarray(ln0_g)
        common["ln0_b"] = np.ascontiguousarray(ln0_b)
        present.add("ln0_g")
        present.add("ln0_b")
    elif np.any(ln0_g != 1.0):
        common["ln0_g"] = np.ascontiguousarray(ln0_g)
        present.add("ln0_g")
    return {"present": present}, common


def _run(inputs, trace=False, nc=None):
    cfg, common = prepare(inputs)
    if nc is None:
        nc = build_nc(cfg)
    import ml_dtypes
    fp8 = ml_dtypes.float8_e4m3
    import ml_dtypes as _md
    in_maps = []
    xall = np.asarray(inputs["x"], np.float32)
    for b in range(B):
        m = dict(common)
        m["x"] = np.ascontiguousarray(xall[b].astype(_md.bfloat16))
        m["xT8"] = np.ascontiguousarray((xall[b].T * np.float32(XS)).astype(fp8))
        in_maps.append(m)
    res = run_bass_kernel_spmd(nc, in_maps, core_ids=list(range(B)),
                               trace=trace)
    out = np.stack([res.results[b]["out"] for b in range(B)], axis=0)
    return out.astype(np.float32), res


def kernel(**inputs):
    out, _ = _run(inputs, trace=False)
    return out


# revision 52
# speedup vs baseline: 1.0302x; 1.0302x over previous
"""Trainium2 Bass kernel for nn_AttentionBlock (B=8, S=2048, D=512), v2.

Sharding: data-parallel over batch B across the 8 NeuronCores (attention is
per-sequence, weights replicated). Each core runs the full block on its own
[S, D] slice; no collectives.

Design (measured on HW at ~217us/core vs 292us for the v1 baseline):
  - host ships xT8 = (8*x)^T as fp8 e4m3 [D, S] plus x as bf16 [S, D]:
    no on-device x transposes and no requantize pass. q/k/v project
    straight out of xT8 with fp8 DoubleRow matmuls.
  - qT/kT live in one merged tile qk[P, KT, 2, S] at 16x scale; q+k psums
    share one [P, 2, SCW] psum pair-tile (2 banks) and drain in a single
    [P, 1024] instruction (ACT and DVE alternating by dt).
  - scores for a tt-pair accumulate into a [P, 2, SCW] psum pair; ONE
    [P, 1024] Exp drains both into the fp8 eT tile (32 exps total).
    All 4 chunks' scores+exps stream back-to-back (eT tiles stay live),
    then the per-chunk tails (attU fp8-DR + DoubleRow ones-matmul row-sum
    + residual + LN0) run, overlapping the FFN start.
  - LN0: DVE reciprocal + scalar_tensor_tensor residual (bf16 out),
    bn_stats/bn_aggr, quake rsqrt batched [P, 4], then one DVE
    tensor_scalar (x-m)*rstd per block writing bf16 onxb.
    (GPSIMD is avoided everywhere: ~4us/instr on real HW.)
  - FFN transposes ride the otherwise-idle DMA engines via
    dma_start_transpose, one [128, 2048] -> [128, 16, 128] XBAR transpose
    per 4-block group (12 total); no psum->sbuf transpose drains at all.
  - LN1/LN2 fold: W2/W3 pre-scaled by gamma on host; the -mean*colsum(W)
    rank-1 correction runs on the PE: mean-rows for two sites come from
    (-1/512 ones-col) @ t matmuls into one small psum, and a K=1 rank-1
    matmul with the host-shipped colsum row closes each site's psum
    accumulation. Two FFN sites share one [P, 2, D] psum pair-tile.
  - FFN pipeline: slot-groups of 4 blocks (s1 g | s2 g-1 | s3 g-2) with
    per-group batched bn stats and rsqrt; fp32 group staging tile and one
    output DMA per group.
  - tensor_tensor_reduce is NOT used (crashes TRN2 HW);
    activation accum_out works but bn_stats is used instead.
"""

import numpy as np
from contextlib import ExitStack

import concourse.bass as bass
import concourse.tile as tile
from concourse import bacc, mybir
from concourse.bass_utils import run_bass_kernel_spmd

# Problem constants (hardcoded per harness contract).
B, S, D = 8, 2048, 512
P = 128
NB = S // P            # 16 row blocks
KT = D // P            # 4 contraction tiles
SCW = 512              # attention s-chunk width
NSC = S // SCW         # 4 chunks
JB = SCW // P          # 4 s-blocks per chunk
GRP = 4                # FFN group size (blocks per pipeline slot-group)
NG = NB // GRP         # 4 groups per stage
EPS = 1e-5
SMSCALE = 1.0 / float(np.sqrt(D))   # BETA=1.0

F32 = mybir.dt.float32
BF16 = mybir.dt.bfloat16
F8 = mybir.dt.float8e4
I32 = mybir.dt.int32
AF = mybir.ActivationFunctionType
ALU = mybir.AluOpType
DR = mybir.MatmulPerfMode.DoubleRow
RSQRT_MAGIC = 0x5F3759DF
# fp8 scaling: x ships as 8*x (both as xT8), W{q,k,v} as 512*W; projection
# psums carry 4096x; qT/kT/v all store 16x values (RS rescale), so softmax
# numerator and the 16x ones-denominator cancel exactly.
QS = 16.0
XS = 8.0
WS = 512.0
RS = QS / (XS * WS)
ESC = SMSCALE / (QS * QS)

WNAMES = ["Wq", "Wk", "Wv", "W1", "W2", "W3"]


def _bcast_ap(ap, parts):
    """[D] dram AP -> [parts, D] AP broadcast along partitions."""
    return bass.AP(tensor=ap.tensor, offset=ap.offset, ap=[[0, parts]] + ap.ap)


def _emit(ctx, tc, cfg, loop_n=1, phases=3):
    nc = tc.nc
    present = cfg["present"]

    # ---- DRAM I/O ----
    x = nc.dram_tensor("x", [S, D], BF16, kind="ExternalInput").ap()
    xT8 = nc.dram_tensor("xT8", [D, S], F8, kind="ExternalInput").ap()
    w_ap = {n: nc.dram_tensor(n, [D, D],
                              F8 if n in ("Wq", "Wk", "Wv") else BF16,
                              kind="ExternalInput").ap()
            for n in WNAMES}
    vec_ap = {}
    for n in ["w2s", "w3s"]:
        vec_ap[n] = nc.dram_tensor(n, [D], BF16, kind="ExternalInput").ap()
    for n in ["bq", "bk", "bv", "b1", "bb2", "bb3", "ln0_g", "ln0_b"]:
        if n in present:
            vec_ap[n] = nc.dram_tensor(n, [D], F32, kind="ExternalInput").ap()
    out = nc.dram_tensor("out", [S, D], F32, kind="ExternalOutput").ap()

    # ---- pools ----
    consts = ctx.enter_context(tc.tile_pool(name="consts", bufs=1))
    wpool = ctx.enter_context(tc.tile_pool(name="wpool", bufs=6))
    bigp = ctx.enter_context(tc.tile_pool(name="big", bufs=1))
    xep = ctx.enter_context(tc.tile_pool(name="xe", bufs=4))
    xld = ctx.enter_context(tc.tile_pool(name="xld", bufs=4))
    work = ctx.enter_context(tc.tile_pool(name="work", bufs=4))
    glp = ctx.enter_context(tc.tile_pool(name="glp", bufs=2))
    soutp = ctx.enter_context(tc.tile_pool(name="soutp", bufs=2))
    ttp = ctx.enter_context(tc.tile_pool(name="ttp", bufs=8))
    small = ctx.enter_context(tc.tile_pool(name="small", bufs=4))
    psb = ctx.enter_context(tc.tile_pool(name="psb", bufs=2, space="PSUM"))
    psa = ctx.enter_context(tc.tile_pool(name="psa", bufs=1, space="PSUM"))
    pss = ctx.enter_context(tc.tile_pool(name="pss", bufs=1, space="PSUM"))

    # psum pair-tile cycler: rotates over 3 [P,2,512] tiles (psb holds 2,
    # psa 1) so every producer->drain pipeline runs 3 deep instead of 2
    _pcycle = [0]

    def pair_alloc(nm):
        _pcycle[0] += 1
        if _pcycle[0] % 3 == 0:
            return psa.tile([P, 2, SCW], F32, tag="mm2", name=nm)
        return psb.tile([P, 2, SCW], F32, tag="mm", name=nm)

    # ---- constants ----
    ones_f = consts.tile([P, 4], F32)
    nc.vector.memset(ones_f[:], QS)
    ones_q = consts.tile([P, 2], F8)
    nc.vector.tensor_copy(ones_q[:], ones_f[:, 0:2])
    ones_q4 = consts.tile([P, 2, 2], F8)
    nc.vector.tensor_copy(ones_q4[:], ones_f[:].rearrange("p (a b) -> p a b",
                                                          a=2))
    onescol = consts.tile([P, 1], BF16)
    nc.vector.memset(onescol[:], -1.0 / D)
    w2sr = consts.tile([1, D], BF16)
    w3sr = consts.tile([1, D], BF16)
    warm = consts.tile([P, 2], F32)
    nc.scalar.activation(warm[:], ones_f[:, 0:2], AF.Exp)
    eps_ap = consts.tile([P, 1], F32)
    nc.vector.memset(eps_ap[:], EPS)

    pp_bias = {}
    for n in ["bq", "bk"]:
        if n in present:
            t = consts.tile([P, KT], F32, tag=f"pp_{n}", name=f"pp_{n}")
            pp_bias[n] = t
    bc_tile = {}
    for n in ["bv", "b1", "bb2", "bb3", "ln0_g", "ln0_b"]:
        if n in present:
            t = consts.tile([P, D], F32, tag=f"bc_{n}", name=f"bc_{n}")
            bc_tile[n] = t

    def load_const_vecs():
        nc.sync.dma_start(w2sr[:], bass.AP(tensor=vec_ap["w2s"].tensor,
                                           offset=vec_ap["w2s"].offset,
                                           ap=[[0, 1]] + vec_ap["w2s"].ap))
        nc.sync.dma_start(w3sr[:], bass.AP(tensor=vec_ap["w3s"].tensor,
                                           offset=vec_ap["w3s"].offset,
                                           ap=[[0, 1]] + vec_ap["w3s"].ap))
        for n, t in pp_bias.items():
            nc.sync.dma_start(t[:], vec_ap[n].rearrange("(kt p) -> p kt", p=P))
        for n, t in bc_tile.items():
            nc.sync.dma_start(t[:], _bcast_ap(vec_ap[n], P))

    # ---- persistent per-sequence tensors ----
    xT8sb = bigp.tile([P, KT, S], F8, tag="xT8sb")
    qk = bigp.tile([P, KT, 2, S], F8, tag="qk")
    vt = bigp.tile([P, NB, D], F8, tag="v")
    onxb_t = [bigp.tile([P, 4, D], BF16, tag=f"onxb{i}", name=f"onxb{i}")
              for i in range(NB // 4)]

    def onxb_ap(n):
        return onxb_t[n // 4][:, n % 4, :]

    def load_w(name):
        dt_ = F8 if name in ("Wq", "Wk", "Wv") else BF16
        wt = wpool.tile([P, KT, D], dt_, tag="w", name=f"w_{name}")
        nc.sync.dma_start(wt[:], w_ap[name].rearrange("(kt p) d -> p kt d", p=P))
        return wt

    def emit_rsqrt(dst, src_ap, n):
        """dst[P,n] = 1/sqrt(src + EPS), DVE-only quake + 2 Newton steps."""
        vps = small.tile([P, n], F32, tag=f"rsq_v{n}", name="rsq_v")
        nc.vector.tensor_scalar_add(vps[:], src_ap, EPS)
        nc.vector.tensor_scalar(dst.bitcast(I32), vps[:].bitcast(I32),
                                1, None, op0=ALU.arith_shift_right)
        nc.vector.tensor_scalar(dst.bitcast(I32), dst.bitcast(I32),
                                -1, RSQRT_MAGIC, op0=ALU.mult, op1=ALU.add)
        t2 = small.tile([P, n], F32, tag=f"rsq_t{n}", name="rsq_t")
        for _ in range(3):
            nc.vector.tensor_tensor(t2[:], dst, dst, op=ALU.mult)
            nc.vector.tensor_tensor(t2[:], t2[:], vps[:], op=ALU.mult)
            nc.vector.tensor_scalar(t2[:], t2[:], -0.5, 1.5,
                                    op0=ALU.mult, op1=ALU.add)
            nc.vector.tensor_tensor(dst, dst, t2[:], op=ALU.mult)

    def emit_rsqrt_act(dst, src_ap):
        """dst = 1/sqrt(src+EPS) via ACT Ln -> Exp(-0.5*). Both funcs live in
        the natural_log_exp table set together with the softmax Exp, so this
        costs no activation-table reload inside the attention region."""
        lnv = small.tile([P, dst.shape[-1]], F32, tag="lnv", name="lnv")
        nc.scalar.activation(lnv[:], src_ap, AF.Ln, bias=eps_ap[:])
        nc.scalar.activation(dst, lnv[:], AF.Exp, scale=-0.5)

    # ================= Phase 1: projections off host-shipped xT8 ======
    if loop_n > 1:
        loop_cm = tc.For_i(0, loop_n, 1)
        loop_cm.__enter__()

    xT8_src = xT8.rearrange("(kt p) s -> p kt s", p=P)
    nc.sync.dma_start(xT8sb[:, :, 0:SCW], xT8_src[:, :, 0:SCW])
    wq = load_w("Wq")
    wk = load_w("Wk")
    nc.sync.dma_start(xT8sb[:, :, SCW:S], xT8_src[:, :, SCW:S])
    wv = load_w("Wv")
    xrgs = []
    for sc in range(NSC):
        xrg = xld.tile([P, JB, D], BF16, tag="xld", name="xrg")
        n0 = sc * JB
        nc.sync.dma_start(
            xrg[:], x[n0 * P:(n0 + JB) * P, :].rearrange(
                "(a p) d -> p a d", p=P))
        xrgs.append(xrg)
    w1 = load_w("W1")   # host: W1 + I (and ln0_g fold when present)
    w2 = load_w("W2")   # host: diag(ln1_g) @ W2
    w3 = load_w("W3")   # host: diag(ln2_g) @ W3
    load_const_vecs()

    def phase1_chunk(sc):
        cs = slice(sc * SCW, (sc + 1) * SCW)
        for dt in range(KT):
            pqk = pair_alloc("pqk")
            for kt in range(0, KT, 2):
                nc.tensor.matmul(pqk[:, 0, :],
                                 wq[:, kt:kt + 2, dt * P:(dt + 1) * P],
                                 xT8sb[:, kt:kt + 2, cs], start=(kt == 0),
                                 stop=(kt == KT - 2), perf_mode=DR)
            for kt in range(0, KT, 2):
                nc.tensor.matmul(pqk[:, 1, :],
                                 wk[:, kt:kt + 2, dt * P:(dt + 1) * P],
                                 xT8sb[:, kt:kt + 2, cs], start=(kt == 0),
                                 stop=(kt == KT - 2), perf_mode=DR)
            if "bq" in pp_bias or "bk" in pp_bias:
                nc.scalar.activation(qk[:, dt, 0, cs], pqk[:, 0, :],
                                     AF.Identity, scale=RS,
                                     bias=pp_bias["bq"][:, dt:dt + 1])
                nc.vector.scalar_tensor_tensor(
                    qk[:, dt, 1, cs], pqk[:, 1, :], RS,
                    pp_bias["bk"][:, dt:dt + 1].to_broadcast([P, SCW]),
                    op0=ALU.mult, op1=ALU.add)
            elif dt % 2 == 0:
                nc.scalar.activation(qk[:, dt, :, cs], pqk[:],
                                     AF.Identity, scale=RS)
            else:
                nc.vector.tensor_scalar_mul(qk[:, dt, :, cs], pqk[:], RS)
        for j in range(0, JB, 2):
            n = sc * JB + j
            pv2 = pair_alloc("pv2")
            for h in range(2):
                for kt in range(0, KT, 2):
                    nc.tensor.matmul(pv2[:, h, :],
                                     xT8sb[:, kt:kt + 2,
                                           (n + h) * P:(n + h + 1) * P],
                                     wv[:, kt:kt + 2, :], start=(kt == 0),
                                     stop=(kt == KT - 2), perf_mode=DR)
            if "bv" in bc_tile:
                for h in range(2):
                    nc.vector.scalar_tensor_tensor(
                        vt[:, n + h, :], pv2[:, h, :], RS,
                        bc_tile["bv"][:], op0=ALU.mult, op1=ALU.add)
            elif j % 4 == 0:
                nc.scalar.activation(vt[:, n:n + 2, :], pv2[:],
                                     AF.Identity, scale=RS)
            else:
                nc.vector.tensor_scalar_mul(vt[:, n:n + 2, :], pv2[:], RS)

    # ================= FFN machinery ==================================
    t1s, t2s, t3s = {}, {}, {}
    rstd1g, rstd2g = {}, {}
    dummy = consts.tile([P, D], BF16, tag="dummy", name="dummy")

    def tpose_group(store, g, src_group):
        """One DMA-transpose for 4 blocks: src [P, 4*D] -> [P, 4*KT, P];
        block i's [P, KT, P] t-tile lives at [:, i*KT:(i+1)*KT, :]."""
        t = ttp.tile([P, GRP * KT, P], BF16, tag="tT", name="tT")
        nc.sync.dma_start_transpose(
            t[:], src_group.rearrange("p a d -> p (a d)"))
        store[g] = t

    def t_block(store, g, i):
        return store[g][:, i * KT:(i + 1) * KT, :]

    def stats_emit(gl, mvf, i):
        # per-site bn stats; variance lands in mvf[:, i, 1]
        st = small.tile([P, 6], F32, tag="fst", name="fst")
        nc.vector.bn_stats(st[:], gl)
        nc.vector.bn_aggr(mvf[:, i, :], st[:])

    def stats_batch(g, store, tagn, mvf):
        rstd = small.tile([P, GRP], F32, tag=tagn, name="rstd")
        emit_rsqrt(rstd[:], mvf[:, :, 1], GRP)
        store[g] = rstd

    def mean_rows2(store, g, i):
        # -mean rows for sites i, i+1 packed in one small psum tile
        pmt = pss.tile([1, 2, P], F32, tag="pmT", name="pmT")
        for h in range(2):
            ts = t_block(store, g, i + h)
            for kt in range(KT):
                nc.tensor.matmul(pmt[:, h, :], onescol[:], ts[:, kt, :],
                                 start=(kt == 0), stop=(kt == KT - 1))
        mT2 = small.tile([1, 2, P], BF16, tag="mT", name="mT")
        nc.scalar.copy(mT2[:], pmt[:])
        return mT2

    _pmpair = [None]

    def ffn_pm_half():
        # two FFN sites share one [P,2,D] psum pair-tile (one bank each)
        if _pmpair[0] is None:
            _pmpair[0] = pair_alloc("pmf")
            return _pmpair[0][:, 0, :]
        h = _pmpair[0][:, 1, :]
        _pmpair[0] = None
        return h

    def ffn_mm2(store, g, i, w, wsr):
        # sites i, i+1: hoisted mean-rows, then W-matmuls + rank-1 each
        mT2 = mean_rows2(store, g, i)
        pmpair = pair_alloc("pmf2")
        pms = []
        for h in range(2):
            ts = t_block(store, g, i + h)
            pm = pmpair[:, h, :]
            for kt in range(KT):
                nc.tensor.matmul(pm, ts[:, kt, :], w[:, kt, :],
                                 start=(kt == 0), stop=False)
            nc.tensor.matmul(pm, mT2[:, h, :], wsr[:], start=False, stop=True)
            pms.append(pm)
        return pms

    def ffn_mm_full(tsrc, w):
        pm = ffn_pm_half()
        for kt in range(KT):
            nc.tensor.matmul(pm, tsrc[:, kt, :], w[:, kt, :],
                             start=(kt == 0), stop=(kt == KT - 1))
        return pm

    def ffn_s1(g, i, glg, mvf):
        pm1 = ffn_mm_full(t_block(t1s, g, i), w1)
        gl = glg[:, i, :]
        if "b1" in bc_tile:
            pre = work.tile([P, D], F32, tag="work", name="pre")
            nc.vector.tensor_add(pre[:], pm1, bc_tile["b1"][:])
            nc.scalar.activation(gl, pre[:], AF.Gelu)
        else:
            nc.scalar.activation(gl, pm1, AF.Gelu)
        stats_emit(gl, mvf, i)

    def ffn_s2pair(g, i, glg, rstd1, mvf):
        pms = ffn_mm2(t2s, g, i, w2, w2sr)
        for h in range(2):
            n = g * GRP + i + h
            pre2 = work.tile([P, D], F32, tag="work", name="pre2")
            nc.vector.scalar_tensor_tensor(pre2[:], pms[h],
                                           rstd1[:, i + h:i + h + 1],
                                           onxb_ap(n), op0=ALU.mult,
                                           op1=ALU.add)
            if "bb2" in bc_tile:
                nc.vector.tensor_add(pre2[:], pre2[:], bc_tile["bb2"][:])
            gl2 = glg[:, i + h, :]
            nc.scalar.activation(gl2, pre2[:], AF.Gelu)
            stats_emit(gl2, mvf, i + h)

    def ffn_s3pair(g, i, soutg, rstd2):
        pms = ffn_mm2(t3s, g, i, w3, w3sr)
        for h in range(2):
            ot = soutg[:, i + h, :]
            if "bb3" in bc_tile:
                nc.vector.scalar_tensor_tensor(ot, pms[h],
                                               rstd2[:, i + h:i + h + 1],
                                               bc_tile["bb3"][:],
                                               op0=ALU.mult, op1=ALU.add)
            else:
                nc.scalar.mul(ot, pms[h], rstd2[:, i + h:i + h + 1])

    def ffn_group(g):
        # s1 over blocks of group g, s2 over g-1, s3 over g-2
        if g < NG:
            mvf = small.tile([P, GRP, 2], F32, tag="mvf1", name="mvf1")
            glg = glp.tile([P, GRP, D], BF16, tag="gl1", name="glg1")
            for i in range(GRP):
                ffn_s1(g, i, glg, mvf)
            tpose_group(t2s, g, glg[:])
            del t1s[g]
            stats_batch(g, rstd1g, "rstd1", mvf)
        if 0 <= g - 1 < NG:
            mvf = small.tile([P, GRP, 2], F32, tag="mvf2", name="mvf2")
            glg = glp.tile([P, GRP, D], BF16, tag="gl2", name="glg2")
            for i in range(0, GRP, 2):
                ffn_s2pair(g - 1, i, glg, rstd1g[g - 1], mvf)
            tpose_group(t3s, g - 1, glg[:])
            del t2s[g - 1]
            stats_batch(g - 1, rstd2g, "rstd2", mvf)
        if 0 <= g - 2 < NG:
            soutg = soutp.tile([P, GRP, D], F32, tag="sout", name="soutg")
            for i in range(0, GRP, 2):
                ffn_s3pair(g - 2, i, soutg, rstd2g[g - 2])
            n0 = (g - 2) * GRP
            nc.sync.dma_start(
                out[n0 * P:(n0 + GRP) * P, :].rearrange("(a p) d -> p a d",
                                                        p=P),
                soutg[:])
            del t3s[g - 2]

    # ================= Phase 2: attention + LN0 =======================
    # software-pipelined chunks: scores+exp of chunk c+1 are emitted before
    # the attU of chunk c so the PE never drains while ACT works the exps
    def emit_scores_part(eT, sc, tp0, tp1):
        cs = slice(sc * SCW, (sc + 1) * SCW)
        for tp in range(tp0, tp1):
            pm2 = pair_alloc("pms")
            for h in range(2):
                tt = 2 * tp + h
                for kt in range(0, KT, 2):
                    nc.tensor.matmul(pm2[:, h, :],
                                     qk[:, kt:kt + 2, 1, tt * P:(tt + 1) * P],
                                     qk[:, kt:kt + 2, 0, cs],
                                     start=(kt == 0), stop=(kt == KT - 2),
                                     perf_mode=DR)
            nc.scalar.activation(eT[:, 2 * tp:2 * tp + 2, :], pm2[:],
                                 AF.Exp, scale=ESC)

    def emit_chunk_tail(sc, eT):
        xrs = [xrgs[sc][:, j, :] for j in range(JB)]
        onxrs = []
        papair = [None]
        for j in range(JB):
            if papair[0] is None:
                papair[0] = pair_alloc("pa")
                pa = papair[0][:, 0, :]
            else:
                pa = papair[0][:, 1, :]
                papair[0] = None
            if phases != 22:
                psm = pss.tile([P, 2], F32, tag="sm", name="psm")
            for tt in range(0, NB, 2):
                nc.tensor.matmul(pa,
                                 eT[:, tt:tt + 2, j * P:(j + 1) * P],
                                 vt[:, tt:tt + 2, :], start=(tt == 0),
                                 stop=(tt == NB - 2), perf_mode=DR)
                if phases == 22:
                    continue
                nc.tensor.matmul(psm[:], eT[:, tt:tt + 2, j * P:(j + 1) * P],
                                 ones_q4[:], start=(tt == 0),
                                 stop=(tt == NB - 2), perf_mode=DR)
            # drain promptly: frees the single psm bank and the pa bank
            rcp = small.tile([P, 1], F32, tag="rcp", name="rcp")
            if phases == 22:
                nc.vector.memset(rcp[:], 1.0)
            else:
                nc.vector.reciprocal(rcp[:], psm[:, 0:1])
            onxr = work.tile([P, D], BF16, tag="wkb", name="onxr")
            nc.vector.scalar_tensor_tensor(onxr[:], pa, rcp[:],
                                           xrs[j],
                                           op0=ALU.mult, op1=ALU.add)
            onxrs.append(onxr)
        if phases in (22, 23):
            return
        mvg = small.tile([P, JB, 2], F32, tag="mvg", name="mvg")
        for j in range(JB):
            st = small.tile([P, 6], F32, tag="bst", name="st")
            nc.vector.bn_stats(st[:], onxrs[j][:])
            nc.vector.bn_aggr(mvg[:, j, :], st[:])
        rstd0 = small.tile([P, JB], F32, tag="rstd0", name="rstd0")
        emit_rsqrt(rstd0[:], mvg[:, :, 1], JB)
        for j in range(JB):
            n = sc * JB + j
            nc.vector.tensor_scalar(onxb_ap(n), onxrs[j][:],
                                    mvg[:, j, 0:1], rstd0[:, j:j + 1],
                                    op0=ALU.subtract, op1=ALU.mult)
            if "ln0_g" in bc_tile:
                nc.vector.tensor_mul(onxb_ap(n), onxb_ap(n),
                                     bc_tile["ln0_g"][:])
            if "ln0_b" in bc_tile:
                nc.vector.tensor_add(onxb_ap(n), onxb_ap(n),
                                     bc_tile["ln0_b"][:])
        # chunk sc == onxb group sc: one group transpose for FFN s1
        tpose_group(t1s, sc, onxb_t[sc][:])

    # chunk-0 scores only need chunk-c k-blocks: interleave their emission
    # with the phase-1 chunks so the exp stream starts ~1/4 into phase 1
    att = phases >= 2 or phases in (21, 22, 23)
    eTs = {}
    if att:
        for sc in range(NSC):
            eTs[sc] = xep.tile([P, NB, SCW], F8, tag="eT", name="eT")
    for sc in range(NSC):
        phase1_chunk(sc)
        if att:
            emit_scores_part(eTs[0], 0, 2 * sc, 2 * sc + 2)
    if att:
        for sc in range(1, NSC):
            emit_scores_part(eTs[sc], sc, 0, NB // 2)
    # tails have no ACT work (exps all live in the scores region), so FFN
    # groups interleave into the tail region with no extra table loads;
    # group g trails tail g+1 so its t1 DMA-transpose latency is hidden
    if phases >= 2 or phases in (22, 23):
        for sc in range(NSC):
            emit_chunk_tail(sc, eTs.pop(sc))
            if phases == 3 and sc >= 1:
                ffn_group(sc - 1)

    # ================= Phase 3: FFN drain =============================
    if phases == 3:
        for g in range(NG - 1, NG + 2):
            ffn_group(g)
    if phases != 3:
        # timing-ablation builds: emit a dummy out store so the output
        # tensor exists
        zt = work.tile([P, D], F32, tag="work", name="zt")
        nc.vector.memset(zt[:], 0.0)
        for n in range(NB):
            nc.sync.dma_start(out[n * P:(n + 1) * P, :], zt[:])
    if loop_n > 1:
        loop_cm.__exit__(None, None, None)


def build_nc(cfg, loop_n=1, phases=3):
    nc = bacc.Bacc("TRN2", target_bir_lowering=False, debug=False)
    with tile.TileContext(nc) as tc:
        with ExitStack() as ctx:
            _emit(ctx, tc, cfg, loop_n=loop_n, phases=phases)
    nc.compile()
    return nc


def prepare(inputs):
    """Host-side folding; returns (cfg, common inputs w/o x, per-core extra)."""
    f32 = np.float32
    import ml_dtypes
    bf16 = ml_dtypes.bfloat16
    fp8 = ml_dtypes.float8_e4m3

    ln0_g = np.asarray(inputs["ln0_g"], f32)
    ln0_b = np.asarray(inputs["ln0_b"], f32)
    ln1_g = np.asarray(inputs["ln1_g"], f32)
    ln1_b = np.asarray(inputs["ln1_b"], f32)
    ln2_g = np.asarray(inputs["ln2_g"], f32)
    ln2_b = np.asarray(inputs["ln2_b"], f32)

    # device computes z = pure LN0; fold gamma into W1' = diag(g)(W1 + I)
    W1p = (ln0_g[:, None] * (np.asarray(inputs["W1"], f32)
                             + np.eye(D, dtype=f32))).astype(bf16)
    W2p = (ln1_g[:, None] * np.asarray(inputs["W2"], f32)).astype(bf16)
    W3p = (ln2_g[:, None] * np.asarray(inputs["W3"], f32)).astype(bf16)
    w2s = W2p.astype(np.float64).sum(0).astype(bf16)
    w3s = W3p.astype(np.float64).sum(0).astype(bf16)
    bb2 = (ln1_b.astype(np.float64) @ np.asarray(inputs["W2"], np.float64)
           + np.asarray(inputs["b2"], np.float64)).astype(f32)
    bb3 = (ln2_b.astype(np.float64) @ np.asarray(inputs["W3"], np.float64)
           + np.asarray(inputs["b3"], np.float64)).astype(f32)

    ws = np.float32(WS)
    common = {
        "Wq": np.ascontiguousarray((np.asarray(inputs["Wq"], f32) * ws).astype(fp8)),
        "Wk": np.ascontiguousarray((np.asarray(inputs["Wk"], f32) * ws).astype(fp8)),
        "Wv": np.ascontiguousarray((np.asarray(inputs["Wv"], f32) * ws).astype(fp8)),
        "W1": np.ascontiguousarray(W1p),
        "W2": np.ascontiguousarray(W2p),
        "W3": np.ascontiguousarray(W3p),
        "w2s": np.ascontiguousarray(w2s),
        "w3s": np.ascontiguousarray(w3s),
    }
    present = set()
    for name, val in [("bq", inputs["bq"]), ("bk", inputs["bk"]),
                      ("bv", inputs["bv"]), ("b1", inputs["b1"]),
                      ("bb2", bb2), ("bb3", bb3)]:
        val = np.asarray(val, f32)
        if np.any(val != 0.0):
            if name in ("bq", "bk", "bv"):
                val = val * np.float32(QS)
            if name == "b1":
                # device h1-pre comes from onxb @ W1p (gamma folded); the
                # b-fold for ln0_b rides bb-style, b1 adds directly
                pass
            common[name] = np.ascontiguousarray(val)
            present.add(name)
    # ln0_b: out_nxt = z*g + b; h1pre = out_nxt @ (I+W1) = z@W1p + b@(I+W1)
    if np.any(ln0_b != 0.0):
        b1fold = (ln0_b.astype(np.float64)
                  @ (np.eye(D) + np.asarray(inputs["W1"], np.float64))
                  ).astype(f32)
        common["b1"] = np.ascontiguousarray(
            common.get("b1", np.zeros(D, f32)) + b1fold)
        present.add("b1")
        # the s2 residual uses onxb (= z); the true residual is z*g + b
        common["ln0_g"] = np.ascontiguousarray(ln0_g)
        common["ln0_b"] = np.ascontiguousarray(ln0_b)
        present.add("ln0_g")
        present.add("ln0_b")
    elif np.any(ln0_g != 1.0):
        common["ln0_g"] = np.ascontiguousarray(ln0_g)
        present.add("ln0_g")
    return {"present": present}, common


def _run(inputs, trace=False, nc=None):
    cfg, common = prepare(inputs)
    if nc is None:
        nc = build_nc(cfg)
    import ml_dtypes
    fp8 = ml_dtypes.float8_e4m3
    import ml_dtypes as _md
    in_maps = []
    xall = np.asarray(inputs["x"], np.float32)
    for b in range(B):
        m = dict(common)
        m["x"] = np.ascontiguousarray(xall[b].astype(_md.bfloat16))
        m["xT8"] = np.ascontiguousarray((xall[b].T * np.float32(XS)).astype(fp8))
        in_maps.append(m)
    res = run_bass_kernel_spmd(nc, in_maps, core_ids=list(range(B)),
                               trace=trace)
    out = np.stack([res.results[b]["out"] for b in range(B)], axis=0)
    return out.astype(np.float32), res


def kernel(**inputs):
    out, _ = _run(inputs, trace=False)
    return out
